# revision 11
# baseline (speedup 1.0000x reference)
"""Trainium2 Bass kernel for nn_Decoder (GNN message passing decoder).

Strategy: receiver-range edge sharding across 8 NeuronCores (no collectives).
Core i owns nodes [NL*i, NL*(i+1)) and every edge whose receiver lands there,
so the segment-sum is core-local. Per core, edges are split into two
sender-index streams (int16 gather limit), grouped by 256-node scatter
windows, and padded to 128-edge chunks with a chunk schedule that is
identical across cores (SPMD: one program, per-core data).

On device (per core):
  - node_attr is cast to a bf16 row-table in SBUF; sender/receiver features
    are fetched with dma_gather(transpose=True) directly into feature-major
    [128f, E] tiles that feed the TensorEngine.
  - Edge MLPs (Le, Me) and sender-grad MLPs (Eg, Sg recomputed per edge) run
    as bf16 matmuls with fp32 PSUM accumulation; SiLU on the scalar engine.
  - Second layers write a packed [8, 512] per-edge scalar block which is
    PE-transposed to [128e, 8]; the 2x2 L/M algebra runs on the vector
    engine; the segment-sum is a one-hot matmul accumulated in PSUM per
    scatter window.
  - Node-local MLPs (Eg, Sg, Ln, Mn) produce the node terms and deg outputs.
Host assembles the per-core [128, 50, 2] outputs into the full result.
"""
import os
import sys
import numpy as np

for _p in ("/opt/trn_rl_repo",):
    if _p not in sys.path:
        sys.path.insert(0, _p)

# ---------------------------------------------------------------- config ---
class Cfg:
    N = 50000          # nodes
    E = 500000         # edges
    H = 128            # feature dim
    NCORES = 8
    NL = 6400          # nodes per core (core 7: 5200 valid)
    WIN = 256          # scatter window (nodes)
    CHUNK = 128        # edges per scatter chunk
    TILE = 512         # edges per compute tile
    SPLIT = 32768      # int16 gather index limit

    @property
    def nwin(self):
        return self.NL // self.WIN

    @property
    def nstripe(self):
        return self.NL // 128


# ------------------------------------------------------------- host prep ---
def build_schedule(cfg, senders, receivers):
    """Chunk schedule (identical across cores) + per-core edge orderings."""
    core_of = receivers // cfg.NL
    r_rel = receivers - core_of * cfg.NL
    stream = (senders >= cfg.SPLIT).astype(np.int64)
    win = r_rel // cfg.WIN

    counts = np.zeros((cfg.NCORES, 2, cfg.nwin), dtype=np.int64)
    np.add.at(counts, (core_of, stream, win), 1)
    chunks_sw = np.maximum(1, -(-counts.max(axis=0) // cfg.CHUNK))
    tpc = cfg.TILE // cfg.CHUNK
    for s in range(2):  # make each stream's chunk count tile-aligned
        chunks_sw[s, cfg.nwin - 1] += (-int(chunks_sw[s].sum())) % tpc

    schedule = []
    for s in range(2):
        for w in range(cfg.nwin):
            c = int(chunks_sw[s, w])
            for j in range(c):
                schedule.append((s, w, j == 0, j == c - 1))
    E_pad = len(schedule) * cfg.CHUNK

    order = np.lexsort((r_rel, win, stream, core_of))
    sc, ss, sw = core_of[order], stream[order], win[order]
    per_core = []
    for i in range(cfg.NCORES):
        perm = np.full(E_pad, -1, dtype=np.int64)
        pos = 0
        for s in range(2):
            for w in range(cfg.nwin):
                idx = order[(sc == i) & (ss == s) & (sw == w)]
                perm[pos:pos + len(idx)] = idx
                pos += int(chunks_sw[s, w]) * cfg.CHUNK
        per_core.append(perm)
    return schedule, per_core, E_pad


def wrap_idx16(idx, num_idxs):
    a = idx.reshape(num_idxs // 16, 16).T
    return np.ascontiguousarray(np.tile(a, (8, 1)).astype(np.int16))


def host_prep(cfg, node_attr, edge_index, edge_attr):
    senders = np.asarray(edge_index[0]).astype(np.int64)
    receivers = np.asarray(edge_index[1]).astype(np.int64)
    schedule, per_core, E_pad = build_schedule(cfg, senders, receivers)

    node_pad = np.zeros((cfg.NCORES * cfg.NL, cfg.H), np.float32)
    node_pad[:cfg.N] = np.asarray(node_attr)

    per_core_inputs = []
    for i in range(cfg.NCORES):
        perm = per_core[i]
        real = perm >= 0
        pidx = np.where(real, perm, 0)
        ea = np.asarray(edge_attr)[pidx] * real[:, None].astype(np.float32)
        s_rel = senders[pidx]
        s_rel = np.where(s_rel >= cfg.SPLIT, s_rel - cfg.SPLIT, s_rel)
        s_rel = np.where(real, s_rel, 0)
        r_loc = np.where(real, receivers[pidx] - i * cfg.NL, 0)
        rw = np.where(real, r_loc % cfg.WIN, 999).astype(np.float32)
        per_core_inputs.append(dict(
            ea_T=np.ascontiguousarray(ea.T.astype(np.float32)),
            sidx=wrap_idx16(s_rel, E_pad),
            ridx=wrap_idx16(r_loc, E_pad),
            rw=np.ascontiguousarray(rw.reshape(-1, cfg.CHUNK).T),
            node_loc=np.ascontiguousarray(
                node_pad[i * cfg.NL:(i + 1) * cfg.NL]),
        ))
    return schedule, E_pad, node_pad, per_core_inputs


def pack_weights(cfg, inputs):
    """Per-MLP packed weights. W2 columns padded into 8 slots:
    slot 0: l (Le/Ln), 1:4: m (Me/Mn), 4:6: gE (Eg), 6:8: gS (Sg)."""
    w = {}
    for p in ("Le", "Me", "Eg", "Sg", "Ln", "Mn"):
        w[p + "_W1"] = np.asarray(inputs[p + "_W1"], np.float32)
        w[p + "_b1"] = np.asarray(inputs[p + "_b1"], np.float32).reshape(-1, 1)
        w2 = np.asarray(inputs[p + "_W2"], np.float32)
        pad = np.zeros((cfg.H, 8), np.float32)
        lo = {"Le": 0, "Ln": 0, "Me": 1, "Mn": 1, "Eg": 4, "Sg": 6}[p]
        pad[:, lo:lo + w2.shape[1]] = w2
        w[p + "_W2p"] = pad
    b2e = np.zeros(8, np.float32)
    b2n = np.zeros(8, np.float32)
    for p, lo, tgt in (("Le", 0, b2e), ("Me", 1, b2e), ("Eg", 4, b2e),
                       ("Sg", 6, b2e), ("Ln", 0, b2n), ("Mn", 1, b2n),
                       ("Eg", 4, b2n), ("Sg", 6, b2n)):
        b2 = np.asarray(inputs[p + "_b2"], np.float32)
        tgt[lo:lo + b2.size] = b2
    w["b2row_e"] = np.tile(b2e, (128, 1))
    w["b2row_n"] = np.tile(b2n, (128, 1))
    return w


# ----------------------------------------------------------- bass kernel ---
def build_nc(cfg, schedule, E_pad):
    import concourse.bass as bass
    import concourse.bacc as bacc
    import concourse.mybir as mybir
    import concourse.tile as tile

    f32 = mybir.dt.float32
    bf16 = mybir.dt.bfloat16
    i16 = mybir.dt.int16
    i32 = mybir.dt.int32
    AF = mybir.ActivationFunctionType
    ALU = mybir.AluOpType

    H = cfg.H
    NTOT = cfg.NCORES * cfg.NL          # padded node table size
    NSTRIPE_TAB = NTOT // 128           # 400
    NST = cfg.nstripe                   # 50 local stripes
    NT = E_pad // cfg.TILE              # edge tiles
    TPC = cfg.TILE // cfg.CHUNK         # chunks per tile

    nc = bacc.Bacc("TRN2", target_bir_lowering=False, debug=False,
                   num_devices=cfg.NCORES)

    # ---- I/O ----
    def din(name, shape, dtype):
        return nc.declare_dram_parameter(name, shape, dtype, isOutput=False)

    node_pad = din("node_pad", [NTOT, H], f32)
    node_loc = din("node_loc", [cfg.NL, H], f32)
    ea_T = din("ea_T", [128, E_pad], f32)
    sidx = din("sidx", [128, E_pad // 16], i16)
    ridx = din("ridx", [128, E_pad // 16], i16)
    rw_in = din("rw", [128, E_pad // cfg.CHUNK], f32)
    wts = {}
    for p in ("Le", "Me", "Eg", "Sg", "Ln", "Mn"):
        fin = 3 * H if p in ("Le", "Me") else H
        wts[p + "_W1"] = din(p + "_W1", [fin, H], f32)
        wts[p + "_b1"] = din(p + "_b1", [H, 1], f32)
        wts[p + "_W2p"] = din(p + "_W2p", [H, 8], f32)
    b2row_e = din("b2row_e", [128, 8], f32)
    b2row_n = din("b2row_n", [128, 8], f32)
    outs = {}
    for nm in ("dz", "dE", "dS"):
        outs[nm] = nc.declare_dram_parameter(nm + "_out", [128, NST, 2], f32,
                                             isOutput=True)

    with tile.TileContext(nc) as tc:
        with (
            tc.tile_pool(name="resident", bufs=1) as rp,
            tc.tile_pool(name="work", bufs=3) as wp,
            tc.tile_pool(name="hsil", bufs=5) as hp,
            tc.tile_pool(name="psum_h", bufs=2, space="PSUM") as ph,
            tc.tile_pool(name="psum_s", bufs=2, space="PSUM") as ps,
            tc.tile_pool(name="psum_w", bufs=2, space="PSUM") as pw,
        ):
            # ---------------- setup: constants and weights ----------------
            tab_rows = rp.tile([128, NSTRIPE_TAB * H], bf16, tag="tab_rows")
            tab_loc = rp.tile([128, NST * H], bf16, tag="tab_loc")
            acc = rp.tile([2, cfg.NL], f32, tag="acc")
            gradsL = rp.tile([128, NST, 8], f32, tag="gradsL")

            iota_i = wp.tile([128, 256], i32, tag="ioti")
            nc.gpsimd.iota(iota_i[:, :], pattern=[[1, 256]],
                           channel_multiplier=0)
            iota_row = rp.tile([128, 256], f32, tag="iotar")
            nc.vector.tensor_copy(iota_row[:, :], iota_i[:, :])
            iota_ci = wp.tile([128, 1], i32, tag="iotci")
            nc.gpsimd.iota(iota_ci[:, :], pattern=[[1, 1]],
                           channel_multiplier=1)
            iota_col = rp.tile([128, 1], f32, tag="iotac")
            nc.vector.tensor_copy(iota_col[:, :], iota_ci[:, :])
            ident_b = rp.tile([128, 128], bf16, tag="identb")
            nc.vector.tensor_scalar(ident_b[:, :], iota_row[:, :128],
                                    iota_col[:, :], None, ALU.is_equal)
            ident_f = rp.tile([128, 128], f32, tag="identf")
            nc.vector.tensor_scalar(ident_f[:, :], iota_row[:, :128],
                                    iota_col[:, :], None, ALU.is_equal)

            W1 = {}
            W2 = {}
            B1 = {}
            for p in ("Le", "Me", "Eg", "Sg", "Ln", "Mn"):
                nch = 3 if p in ("Le", "Me") else 1
                for c in range(nch):
                    t32 = wp.tile([128, H], f32, tag="wld")
                    nc.sync.dma_start(t32[:, :],
                                      wts[p + "_W1"][c * H:(c + 1) * H, :])
                    tb = rp.tile([128, H], bf16, tag=f"W1_{p}_{c}")
                    nc.vector.tensor_copy(tb[:, :], t32[:, :])
                    W1[p, c] = tb
                t32 = wp.tile([128, 8], f32, tag="wld2")
                nc.sync.dma_start(t32[:, :], wts[p + "_W2p"][:, :])
                tb = rp.tile([128, 8], bf16, tag=f"W2_{p}")
                nc.vector.tensor_copy(tb[:, :], t32[:, :])
                W2[p] = tb
                tb1 = rp.tile([H, 1], f32, tag=f"b1_{p}")
                nc.sync.dma_start(tb1[:, :], wts[p + "_b1"][:, :])
                B1[p] = tb1
            b2e_t = rp.tile([128, 8], f32, tag="b2e")
            nc.sync.dma_start(b2e_t[:, :], b2row_e[:, :])
            b2n_t = rp.tile([128, 8], f32, tag="b2n")
            nc.sync.dma_start(b2n_t[:, :], b2row_n[:, :])

            nc.vector.memset(acc[:, :], 0.0)

            # node table loads (f32 -> bf16 cast during SWDGE DMA)
            np_v = node_pad.rearrange("(s p) f -> p s f", p=128)
            tr_v = tab_rows[:, :].rearrange("p (s f) -> p s f", f=H)
            spc = 100  # stripes per cast-DMA (12800 descriptors < 16384)
            for s0 in range(0, NSTRIPE_TAB, spc):
                s1 = min(s0 + spc, NSTRIPE_TAB)
                nc.gpsimd.dma_start(tr_v[:, s0:s1, :], np_v[:, s0:s1, :])
            nl_v = node_loc.rearrange("(s p) f -> p s f", p=128)
            nc.gpsimd.dma_start(
                tab_loc[:, :].rearrange("p (s f) -> p s f", f=H), nl_v)

            # ---------------- phase 1: node-local MLPs ----------------
            groups = []
            s0 = 0
            while s0 < NST:
                gw = min(4, NST - s0)
                groups.append((s0, gw))
                s0 += gw
            for (s0, gw) in groups:
                wd = gw * 128
                tabT = wp.tile([128, 512], bf16, tag="tabT")
                for k in range(gw):
                    ptr = ps.tile([128, 128], bf16, tag="tr")
                    nc.tensor.transpose(
                        ptr[:, :],
                        tab_loc[:, (s0 + k) * H:(s0 + k + 1) * H],
                        ident_b[:, :])
                    nc.vector.tensor_copy(tabT[:, k * 128:(k + 1) * 128],
                                          ptr[:, :])
                hs = {}
                for p in ("Eg", "Sg", "Ln", "Mn"):
                    hps = ph.tile([128, 512], f32, tag="hps")
                    nc.tensor.matmul(hps[:, :wd], W1[p, 0][:, :],
                                     tabT[:, :wd], start=True, stop=True)
                    hsb = hp.tile([128, 512], bf16, tag="hsb")
                    nc.scalar.activation(hsb[:, :wd], hps[:, :wd], AF.Silu,
                                         bias=B1[p][:, :])
                    hs[p] = hsb
                spsum = ps.tile([8, 512], f32, tag="spsum")
                for j, p in enumerate(("Ln", "Mn", "Eg", "Sg")):
                    nc.tensor.matmul(spsum[:, :wd], W2[p][:, :],
                                     hs[p][:, :wd], start=(j == 0),
                                     stop=(j == 3))
                ssb = wp.tile([8, 512], f32, tag="ssb")
                nc.scalar.copy(ssb[:, :wd], spsum[:, :wd])
                for k in range(gw):
                    ptr8 = ps.tile([128, 8], f32, tag="tr")
                    nc.tensor.transpose(ptr8[:, :],
                                        ssb[:8, k * 128:(k + 1) * 128],
                                        ident_f[:8, :8])
                    nc.vector.tensor_add(gradsL[:, s0 + k, :], ptr8[:, :],
                                         b2n_t[:, :])

            # ---------------- phase 2: edge tiles ----------------
            win_psum = None
            for t in range(NT):
                ea_b = wp.tile([128, cfg.TILE], bf16, tag="ea")
                nc.gpsimd.dma_start(ea_b[:, :],
                                    ea_T[:, t * cfg.TILE:(t + 1) * cfg.TILE])
                six = wp.tile([128, cfg.TILE // 16], i16, tag="six")
                nc.sync.dma_start(
                    six[:, :], sidx[:, t * cfg.TILE // 16:
                                    (t + 1) * cfg.TILE // 16])
                rix = wp.tile([128, cfg.TILE // 16], i16, tag="rix")
                nc.sync.dma_start(
                    rix[:, :], ridx[:, t * cfg.TILE // 16:
                                    (t + 1) * cfg.TILE // 16])
                rwt = wp.tile([128, TPC], f32, tag="rwt")
                nc.sync.dma_start(rwt[:, :], rw_in[:, t * TPC:(t + 1) * TPC])

                stream = schedule[t * TPC][0]
                gs = wp.tile([128, 1, cfg.TILE], bf16, tag="gs")
                src = tab_rows[:, :]
                if stream == 1:
                    src = tab_rows[:, cfg.SPLIT:]
                nc.gpsimd.dma_gather(
                    gs[:, :, :], src, six[:, :], num_idxs=cfg.TILE,
                    num_idxs_reg=cfg.TILE, elem_size=H, transpose=True,
                    sbuf_tokens_per_rank=128, sbuf_free_dim_per_rank=2 * H)
                gr = wp.tile([128, 1, cfg.TILE], bf16, tag="gr")
                nc.gpsimd.dma_gather(
                    gr[:, :, :], tab_loc[:, :], rix[:, :], num_idxs=cfg.TILE,
                    num_idxs_reg=cfg.TILE, elem_size=H, transpose=True,
                    sbuf_tokens_per_rank=128, sbuf_free_dim_per_rank=2 * H)

                hs = {}
                for p in ("Le", "Me"):
                    hps = ph.tile([128, cfg.TILE], f32, tag="hps")
                    nc.tensor.matmul(hps[:, :], W1[p, 0][:, :], ea_b[:, :],
                                     start=True, stop=False)
                    nc.tensor.matmul(hps[:, :], W1[p, 1][:, :],
                                     gs[:, 0, :], start=False, stop=False)
                    nc.tensor.matmul(hps[:, :], W1[p, 2][:, :],
                                     gr[:, 0, :], start=False, stop=True)
                    hsb = hp.tile([128, cfg.TILE], bf16, tag="hsb")
                    nc.scalar.activation(hsb[:, :], hps[:, :], AF.Silu,
                                         bias=B1[p][:, :])
                    hs[p] = hsb
                for p in ("Eg", "Sg"):
                    hps = ph.tile([128, cfg.TILE], f32, tag="hps")
                    nc.tensor.matmul(hps[:, :], W1[p, 0][:, :], gs[:, 0, :],
                                     start=True, stop=True)
                    hsb = hp.tile([128, cfg.TILE], bf16, tag="hsb")
                    nc.scalar.activation(hsb[:, :], hps[:, :], AF.Silu,
                                         bias=B1[p][:, :])
                    hs[p] = hsb
                spsum = ps.tile([8, 512], f32, tag="spsum")
                for j, p in enumerate(("Le", "Me", "Eg", "Sg")):
                    nc.tensor.matmul(spsum[:, :], W2[p][:, :], hs[p][:, :],
                                     start=(j == 0), stop=(j == 3))
                ssb = wp.tile([8, 512], f32, tag="ssb")
                nc.scalar.copy(ssb[:, :], spsum[:, :])

                Tps = ps.tile([128, TPC, 8], f32, tag="tr")
                for k in range(TPC):
                    nc.tensor.transpose(Tps[:, k, :],
                                        ssb[:8, k * 128:(k + 1) * 128],
                                        ident_f[:8, :8])
                # bias add + term algebra (fp32, e on partitions)
                S = wp.tile([128, TPC, 8], f32, tag="S")
                for k in range(TPC):
                    nc.vector.tensor_add(S[:, k, :], Tps[:, k, :],
                                         b2e_t[:, :])
                def col(v):
                    return S[:, :, v:v + 1]
                u = wp.tile([128, TPC, 1], f32, tag="u")
                p1 = wp.tile([128, TPC, 1], f32, tag="p1")
                term = wp.tile([128, TPC, 2], f32, tag="term")
                nc.vector.tensor_mul(u[:, :, :], col(1), col(6))
                nc.vector.tensor_mul(p1[:, :, :], col(2), col(7))
                nc.vector.tensor_add(u[:, :, :], u[:, :, :], p1[:, :, :])
                # t0 = m0*u - l*gE1
                nc.vector.tensor_mul(term[:, :, 0:1], col(1), u[:, :, :])
                nc.vector.tensor_mul(p1[:, :, :], col(0), col(5))
                nc.vector.tensor_sub(term[:, :, 0:1], term[:, :, 0:1],
                                     p1[:, :, :])
                # t1 = l*gE0 + m1*u + m2^2*gS1
                nc.vector.tensor_mul(term[:, :, 1:2], col(2), u[:, :, :])
                nc.vector.tensor_mul(p1[:, :, :], col(0), col(4))
                nc.vector.tensor_add(term[:, :, 1:2], term[:, :, 1:2],
                                     p1[:, :, :])
                nc.vector.tensor_mul(p1[:, :, :], col(3), col(3))
                nc.vector.tensor_mul(p1[:, :, :], p1[:, :, :], col(7))
                nc.vector.tensor_add(term[:, :, 1:2], term[:, :, 1:2],
                                     p1[:, :, :])
                term_b = wp.tile([128, TPC, 2], bf16, tag="termb")
                nc.vector.tensor_copy(term_b[:, :, :], term[:, :, :])

                for k in range(TPC):
                    s_, w_, st_, sp_ = schedule[t * TPC + k]
                    U = wp.tile([128, cfg.WIN], bf16, tag="U")
                    nc.vector.tensor_scalar(U[:, :], iota_row[:, :cfg.WIN],
                                            rwt[:, k:k + 1], None,
                                            ALU.is_equal)
                    if st_:
                        win_psum = pw.tile([2, cfg.WIN], f32, tag="wps")
                    nc.tensor.matmul(win_psum[:, :], term_b[:, k, :],
                                     U[:, :], start=st_, stop=sp_)
                    if sp_:
                        a_sl = acc[:, w_ * cfg.WIN:(w_ + 1) * cfg.WIN]
                        nc.vector.tensor_add(a_sl, a_sl, win_psum[:, :])

            # ---------------- phase 3: finalize ----------------
            dzs = rp.tile([128, NST, 2], f32, tag="dzs")
            dEs = rp.tile([128, NST, 2], f32, tag="dEs")
            dSs = rp.tile([128, NST, 2], f32, tag="dSs")
            for (s0, gw) in groups:
                accT = ps.tile([128, 4, 2], f32, tag="tr")
                for k in range(gw):
                    nc.tensor.transpose(
                        accT[:, k, :],
                        acc[:, (s0 + k) * 128:(s0 + k + 1) * 128],
                        ident_f[:2, :2])
                G = gradsL[:, s0:s0 + gw, :]

                def gcol(v):
                    return G[:, :, v:v + 1]
                u = wp.tile([128, 4, 1], f32, tag="fu")
                p1 = wp.tile([128, 4, 1], f32, tag="fp1")
                us = u[:, :gw, :]
                p1s = p1[:, :gw, :]
                # u = mn0*gS0 + mn1*gS1
                nc.vector.tensor_mul(us, gcol(1), gcol(6))
                nc.vector.tensor_mul(p1s, gcol(2), gcol(7))
                nc.vector.tensor_add(us, us, p1s)
                dz_sl = dzs[:, s0:s0 + gw, :]
                # dz0 = mn0*u - ln*gE1 - acc0
                nc.vector.tensor_mul(dz_sl[:, :, 0:1], gcol(1), us)
                nc.vector.tensor_mul(p1s, gcol(0), gcol(5))
                nc.vector.tensor_sub(dz_sl[:, :, 0:1], dz_sl[:, :, 0:1], p1s)
                # dz1 = ln*gE0 + mn1*u + mn2^2*gS1
                nc.vector.tensor_mul(dz_sl[:, :, 1:2], gcol(2), us)
                nc.vector.tensor_mul(p1s, gcol(0), gcol(4))
                nc.vector.tensor_add(dz_sl[:, :, 1:2], dz_sl[:, :, 1:2], p1s)
                nc.vector.tensor_mul(p1s, gcol(3), gcol(3))
                nc.vector.tensor_mul(p1s, p1s, gcol(7))
                nc.vector.tensor_add(dz_sl[:, :, 1:2], dz_sl[:, :, 1:2], p1s)
                nc.vector.tensor_sub(dz_sl, dz_sl, accT[:, :gw, :])
                # vE = mn0*gE0 + mn1*gE1 ; dE0 = mn0*vE ;
                # dE1 = mn1*vE + mn2^2*gE1
                dE_sl = dEs[:, s0:s0 + gw, :]
                nc.vector.tensor_mul(us, gcol(1), gcol(4))
                nc.vector.tensor_mul(p1s, gcol(2), gcol(5))
                nc.vector.tensor_add(us, us, p1s)
                nc.vector.tensor_mul(dE_sl[:, :, 0:1], gcol(1), us)
                nc.vector.tensor_mul(dE_sl[:, :, 1:2], gcol(2), us)
                nc.vector.tensor_mul(p1s, gcol(3), gcol(3))
                nc.vector.tensor_mul(p1s, p1s, gcol(5))
                nc.vector.tensor_add(dE_sl[:, :, 1:2], dE_sl[:, :, 1:2], p1s)
                # dS0 = -ln*gS1 ; dS1 = ln*gS0
                dS_sl = dSs[:, s0:s0 + gw, :]
                nc.vector.tensor_mul(p1s, gcol(0), gcol(7))
                nc.vector.memset(dS_sl[:, :, 0:1], 0.0)
                nc.vector.tensor_sub(dS_sl[:, :, 0:1], dS_sl[:, :, 0:1], p1s)
                nc.vector.tensor_mul(dS_sl[:, :, 1:2], gcol(0), gcol(6))
            nc.sync.dma_start(outs["dz"][:, :, :], dzs[:, :, :])
            nc.sync.dma_start(outs["dE"][:, :, :], dEs[:, :, :])
            nc.sync.dma_start(outs["dS"][:, :, :], dSs[:, :, :])
    nc.compile()
    return nc


# ----------------------------------------------------------------- entry ---
def kernel(**inputs):
    from concourse.bass_utils import run_bass_kernel_spmd

    cfg = Cfg()
    schedule, E_pad, node_pad, pci = host_prep(
        cfg, inputs["node_attr"], inputs["edge_index"], inputs["edge_attr"])
    w = pack_weights(cfg, inputs)

    nc = build_nc(cfg, schedule, E_pad)

    in_maps = []
    for i in range(cfg.NCORES):
        m = dict(node_pad=node_pad, b2row_e=w["b2row_e"],
                 b2row_n=w["b2row_n"])
        for p in ("Le", "Me", "Eg", "Sg", "Ln", "Mn"):
            m[p + "_W1"] = w[p + "_W1"]
            m[p + "_b1"] = w[p + "_b1"]
            m[p + "_W2p"] = w[p + "_W2p"]
        m.update(pci[i])
        in_maps.append(m)

    trace = os.environ.get("BASS_KERNEL_TRACE") == "1"
    r = run_bass_kernel_spmd(nc, in_maps, list(range(cfg.NCORES)),
                             trace=trace,
                             trace_cores=list(range(cfg.NCORES)) if trace
                             else None)
    if trace:
        print(f"HW exec time: {r.exec_time_ns} ns  "
              f"(mean {r.mean_exec_time_ns} ns, "
              f"max core {r.max_exec_time_core_id})")
    res = r.results

    dz = np.zeros((cfg.N, 2), np.float32)
    dE = np.zeros((cfg.N, 2), np.float32)
    dS = np.zeros((cfg.N, 2), np.float32)
    for i in range(cfg.NCORES):
        lo = i * cfg.NL
        hi = min((i + 1) * cfg.NL, cfg.N)
        n = hi - lo
        for nm, buf in (("dz_out", dz), ("dE_out", dE), ("dS_out", dS)):
            a = res[i][nm]  # [128, NST, 2]
            buf[lo:hi] = a.transpose(1, 0, 2).reshape(cfg.NL, 2)[:n]
    return (dz, dE[:, :, None], dS[:, :, None])


# revision 29
# speedup vs baseline: 1.0736x; 1.0736x over previous
"""Trainium2 Bass kernel for nn_Decoder (GNN message passing decoder).

Strategy: receiver-range edge sharding across 8 NeuronCores (no collectives).
Core i owns nodes [NL*i, NL*(i+1)) and every edge whose receiver lands there,
so the segment-sum is core-local. Per core, edges are split into two
sender-index streams (int16 gather limit), sorted by receiver, grouped by
128-node scatter windows and padded to 128-edge chunks with a chunk schedule
that is identical across cores (SPMD: one program, per-core data).

On device (per core):
  - node_attr is cast to a bf16 row table in SBUF; sender rows are fetched
    with dma_gather(transpose=True) straight into feature-major [128f, 512e]
    tiles.
  - Receiver features never get gathered: since edges are receiver-sorted,
    each chunk's receivers live in one 128-node window, and the receiver
    contribution to the first layer is P_w @ U^T where P_wT = tabT_w.T @ W1r
    is precomputed per window and U^T is a one-hot built on the fly.
  - Edge MLPs (Le, Me) and sender-grad MLPs (Eg, Sg recomputed per edge) run
    as bf16 matmuls with fp32 PSUM accumulation; SiLU on the scalar engine.
  - Second layers write a packed [8, 512] per-edge scalar block, PE-transposed
    to [128e, 8]; the 2x2 L/M algebra runs on the vector engine (batched over
    4 tiles); the segment-sum is a one-hot matmul accumulated in PSUM per
    scatter window.
  - Node-local MLPs (Eg, Sg, Ln, Mn) produce the node terms and deg outputs.
Host assembles the per-core [128, 50, 2] outputs into the full result.
"""
import os
import sys
import numpy as np

for _p in ("/opt/trn_rl_repo",):
    if _p not in sys.path:
        sys.path.insert(0, _p)

SBATCH = 4  # tiles per term-math batch
DEBUG_TAPS = False


# ---------------------------------------------------------------- config ---
class Cfg:
    N = 50000          # nodes
    E = 500000         # edges
    H = 128            # feature dim
    NCORES = 8
    NL = 6400          # nodes per core (core 7: 5200 valid)
    WIN = 128          # scatter window (nodes)
    CHUNK = 128        # edges per scatter chunk
    TILE = 512         # edges per compute tile
    SPLIT = 32768      # int16 gather index limit

    @property
    def nwin(self):
        return self.NL // self.WIN

    @property
    def nstripe(self):
        return self.NL // 128


# ------------------------------------------------------------- host prep ---
def build_schedule(cfg, senders, receivers):
    """Chunk schedule (identical across cores) + per-core edge orderings."""
    core_of = receivers // cfg.NL
    r_rel = receivers - core_of * cfg.NL
    stream = (senders >= cfg.SPLIT).astype(np.int64)
    win = r_rel // cfg.WIN

    counts = np.zeros((cfg.NCORES, 2, cfg.nwin), dtype=np.int64)
    np.add.at(counts, (core_of, stream, win), 1)
    chunks_sw = np.maximum(1, -(-counts.max(axis=0) // cfg.CHUNK))
    tpc = cfg.TILE // cfg.CHUNK
    # stream 0 tile-aligned; whole schedule aligned to SBATCH tiles
    chunks_sw[0, cfg.nwin - 1] += (-int(chunks_sw[0].sum())) % tpc
    grp = SBATCH * tpc
    chunks_sw[1, cfg.nwin - 1] += (-int(chunks_sw.sum())) % grp

    schedule = []
    for s in range(2):
        for w in range(cfg.nwin):
            c = int(chunks_sw[s, w])
            for j in range(c):
                schedule.append((s, w, j == 0, j == c - 1))
    E_pad = len(schedule) * cfg.CHUNK

    order = np.lexsort((r_rel, win, stream, core_of))
    sc, ss, sw = core_of[order], stream[order], win[order]
    per_core = []
    for i in range(cfg.NCORES):
        perm = np.full(E_pad, -1, dtype=np.int64)
        pos = 0
        for s in range(2):
            for w in range(cfg.nwin):
                idx = order[(sc == i) & (ss == s) & (sw == w)]
                perm[pos:pos + len(idx)] = idx
                pos += int(chunks_sw[s, w]) * cfg.CHUNK
        per_core.append(perm)
    return schedule, per_core, E_pad


def wrap_idx16(idx, num_idxs):
    a = idx.reshape(num_idxs // 16, 16).T
    return np.ascontiguousarray(np.tile(a, (8, 1)).astype(np.int16))


def host_prep(cfg, node_attr, edge_index, edge_attr):
    senders = np.asarray(edge_index[0]).astype(np.int64)
    receivers = np.asarray(edge_index[1]).astype(np.int64)
    schedule, per_core, E_pad = build_schedule(cfg, senders, receivers)

    node_pad = np.zeros((cfg.NCORES * cfg.NL, cfg.H), np.float32)
    node_pad[:cfg.N] = np.asarray(node_attr)

    per_core_inputs = []
    for i in range(cfg.NCORES):
        perm = per_core[i]
        real = perm >= 0
        pidx = np.where(real, perm, 0)
        ea = np.asarray(edge_attr)[pidx] * real[:, None].astype(np.float32)
        s_rel = senders[pidx]
        s_rel = np.where(s_rel >= cfg.SPLIT, s_rel - cfg.SPLIT, s_rel)
        s_rel = np.where(real, s_rel, 0)
        rw = np.where(real, (receivers[pidx] - i * cfg.NL) % cfg.WIN,
                      999).astype(np.float32)
        per_core_inputs.append(dict(
            ea_T=np.ascontiguousarray(ea.T.astype(np.float32)),
            sidx=wrap_idx16(s_rel, E_pad),
            rw=np.ascontiguousarray(rw.reshape(-1, cfg.CHUNK).T),
            rwr=np.ascontiguousarray(rw.reshape(1, -1)),
            node_loc=np.ascontiguousarray(
                node_pad[i * cfg.NL:(i + 1) * cfg.NL]),
        ))
    return schedule, E_pad, node_pad, per_core_inputs


def pack_weights(cfg, inputs):
    """Per-MLP packed weights. W2 columns padded into 8 slots:
    slot 0: l (Le/Ln), 1:4: m (Me/Mn), 4:6: gE (Eg), 6:8: gS (Sg)."""
    w = {}
    for p in ("Le", "Me", "Eg", "Sg", "Ln", "Mn"):
        w[p + "_W1"] = np.asarray(inputs[p + "_W1"], np.float32)
        w[p + "_b1"] = np.asarray(inputs[p + "_b1"], np.float32).reshape(-1, 1)
        w2 = np.asarray(inputs[p + "_W2"], np.float32)
        pad = np.zeros((cfg.H, 8), np.float32)
        lo = {"Le": 0, "Ln": 0, "Me": 1, "Mn": 1, "Eg": 4, "Sg": 6}[p]
        pad[:, lo:lo + w2.shape[1]] = w2
        w[p + "_W2p"] = pad
    b2e = np.zeros(8, np.float32)
    b2n = np.zeros(8, np.float32)
    for p, lo, tgt in (("Le", 0, b2e), ("Me", 1, b2e), ("Eg", 4, b2e),
                      ("Sg", 6, b2e), ("Ln", 0, b2n), ("Mn", 1, b2n),
                      ("Eg", 4, b2n), ("Sg", 6, b2n)):
        b2 = np.asarray(inputs[p + "_b2"], np.float32)
        tgt[lo:lo + b2.size] = b2
    w["b2row_e"] = np.tile(b2e, (128, 1))
    w["b2row_n"] = np.tile(b2n, (128, 1))
    return w


# ----------------------------------------------------------- bass kernel ---
def build_nc(cfg, schedule, E_pad):
    import concourse.bass as bass
    import concourse.bacc as bacc
    import concourse.mybir as mybir
    import concourse.tile as tile

    f32 = mybir.dt.float32
    bf16 = mybir.dt.bfloat16
    i16 = mybir.dt.int16
    i32 = mybir.dt.int32
    AF = mybir.ActivationFunctionType
    ALU = mybir.AluOpType

    H = cfg.H
    NTOT = cfg.NCORES * cfg.NL          # padded node table size
    NSTRIPE_TAB = NTOT // 128           # 400
    NST = cfg.nstripe                   # 50 local stripes (= windows)
    NT = E_pad // cfg.TILE              # edge tiles
    TPC = cfg.TILE // cfg.CHUNK         # chunks per tile

    nc = bacc.Bacc("TRN2", target_bir_lowering=False, debug=False,
                   num_devices=cfg.NCORES)

    # ---- I/O ----
    def din(name, shape, dtype):
        return nc.declare_dram_parameter(name, shape, dtype, isOutput=False)

    node_pad = din("node_pad", [NTOT, H], f32)
    node_loc = din("node_loc", [cfg.NL, H], f32)
    ea_T = din("ea_T", [128, E_pad], f32)
    sidx = din("sidx", [128, E_pad // 16], i16)
    rw_in = din("rw", [128, E_pad // cfg.CHUNK], f32)
    rwr_in = din("rwr", [1, E_pad], f32)
    wts = {}
    for p in ("Le", "Me", "Eg", "Sg", "Ln", "Mn"):
        fin = 3 * H if p in ("Le", "Me") else H
        wts[p + "_W1"] = din(p + "_W1", [fin, H], f32)
        wts[p + "_b1"] = din(p + "_b1", [H, 1], f32)
        wts[p + "_W2p"] = din(p + "_W2p", [H, 8], f32)
    b2row_e = din("b2row_e", [128, 8], f32)
    b2row_n = din("b2row_n", [128, 8], f32)
    outs = {}
    for nm in ("dz", "dE", "dS"):
        outs[nm] = nc.declare_dram_parameter(nm + "_out", [128, NST, 2], f32,
                                             isOutput=True)
    if DEBUG_TAPS:
        dbg_P = nc.declare_dram_parameter("dbg_P", [128, NST, H], f32,
                                          isOutput=True)
        dbg_UT = nc.declare_dram_parameter("dbg_UT", [128, cfg.TILE], f32,
                                           isOutput=True)
        dbg_h = nc.declare_dram_parameter("dbg_h", [128, cfg.TILE], f32,
                                          isOutput=True)

    with tile.TileContext(nc) as tc:
        with (
            tc.tile_pool(name="resident", bufs=1) as rp,
            tc.tile_pool(name="work", bufs=3) as wp,
            tc.tile_pool(name="hsil", bufs=4) as hp,
            tc.tile_pool(name="psum_h", bufs=2, space="PSUM") as ph,
            tc.tile_pool(name="psum_s", bufs=1, space="PSUM") as ps,
            tc.tile_pool(name="psum_b", bufs=2, space="PSUM") as pb,
            tc.tile_pool(name="psum_w", bufs=2, space="PSUM") as pw,
        ):
            # ---------------- setup: constants and weights ----------------
            tab_rows = rp.tile([128, NSTRIPE_TAB * H], bf16, tag="tab_rows")
            tab_loc = rp.tile([128, NST * H], bf16, tag="tab_loc")
            acc = rp.tile([2, cfg.NL], f32, tag="acc")
            gradsL = rp.tile([128, NST, 8], f32, tag="gradsL")
            PLe = rp.tile([128, NST, H], bf16, tag="PLe")
            PMe = rp.tile([128, NST, H], bf16, tag="PMe")

            iota_i = wp.tile([128, 256], i32, tag="ioti")
            nc.gpsimd.iota(iota_i[:, :], pattern=[[1, 256]],
                           channel_multiplier=0)
            iota_row = rp.tile([128, 256], f32, tag="iotar")
            nc.vector.tensor_copy(iota_row[:, :], iota_i[:, :])
            iota_ci = wp.tile([128, 1], i32, tag="iotci")
            nc.gpsimd.iota(iota_ci[:, :], pattern=[[1, 1]],
                           channel_multiplier=1)
            iota_col = rp.tile([128, 1], f32, tag="iotac")
            nc.vector.tensor_copy(iota_col[:, :], iota_ci[:, :])
            ident_b = rp.tile([128, 128], bf16, tag="identb")
            nc.vector.tensor_scalar(ident_b[:, :], iota_row[:, :128],
                                    iota_col[:, :], None, ALU.is_equal)
            ident_f = rp.tile([128, 128], f32, tag="identf")
            nc.vector.tensor_scalar(ident_f[:, :], iota_row[:, :128],
                                    iota_col[:, :], None, ALU.is_equal)
            ones_c = rp.tile([1, 128], f32, tag="onesc")
            nc.vector.memset(ones_c[:, :], 1.0)

            W1 = {}
            W2 = {}
            B1 = {}
            for p in ("Le", "Me", "Eg", "Sg", "Ln", "Mn"):
                nch = 3 if p in ("Le", "Me") else 1
                for c in range(nch):
                    t32 = wp.tile([128, H], f32, tag="wld")
                    nc.sync.dma_start(t32[:, :],
                                      wts[p + "_W1"][c * H:(c + 1) * H, :])
                    tb = rp.tile([128, H], bf16, tag=f"W1_{p}_{c}")
                    nc.vector.tensor_copy(tb[:, :], t32[:, :])
                    W1[p, c] = tb
                t32 = wp.tile([128, 8], f32, tag="wld2")
                nc.sync.dma_start(t32[:, :], wts[p + "_W2p"][:, :])
                tb = rp.tile([128, 8], bf16, tag=f"W2_{p}")
                nc.vector.tensor_copy(tb[:, :], t32[:, :])
                W2[p] = tb
                tb1 = rp.tile([H, 1], f32, tag=f"b1_{p}")
                nc.sync.dma_start(tb1[:, :], wts[p + "_b1"][:, :])
                B1[p] = tb1
            b2e_t = rp.tile([128, 8], f32, tag="b2e")
            nc.sync.dma_start(b2e_t[:, :], b2row_e[:, :])
            b2n_t = rp.tile([128, 8], f32, tag="b2n")
            nc.sync.dma_start(b2n_t[:, :], b2row_n[:, :])

            nc.vector.memset(acc[:, :], 0.0)

            # node tables (f32 -> bf16 cast during SWDGE DMA)
            np_v = node_pad.rearrange("(s p) f -> p s f", p=128)
            tr_v = tab_rows[:, :].rearrange("p (s f) -> p s f", f=H)
            spc = min(100, NSTRIPE_TAB)
            for s0 in range(0, NSTRIPE_TAB, spc):
                s1 = min(s0 + spc, NSTRIPE_TAB)
                nc.gpsimd.dma_start(tr_v[:, s0:s1, :], np_v[:, s0:s1, :])
            nl_v = node_loc.rearrange("(s p) f -> p s f", p=128)
            nc.gpsimd.dma_start(
                tab_loc[:, :].rearrange("p (s f) -> p s f", f=H), nl_v)

            # ---------------- phase 1: node-local MLPs + P tables ---------
            groups = []
            s0 = 0
            while s0 < NST:
                gw = min(4, NST - s0)
                groups.append((s0, gw))
                s0 += gw
            for (s0, gw) in groups:
                wd = gw * 128
                tabT = wp.tile([128, 512], bf16, tag="tabT")
                for k in range(gw):
                    ptr = ps.tile([128, 128], bf16, tag="tr")
                    nc.tensor.transpose(
                        ptr[:, :],
                        tab_loc[:, (s0 + k) * H:(s0 + k + 1) * H],
                        ident_b[:, :])
                    nc.vector.tensor_copy(tabT[:, k * 128:(k + 1) * 128],
                                          ptr[:, :])
                # receiver projection tables P_wT = tabT_w.T @ W1r
                for k in range(gw):
                    for p, Pt in (("Le", PLe), ("Me", PMe)):
                        pps = pb.tile([128, H], f32, tag="bq")
                        nc.tensor.matmul(pps[:, :],
                                         tabT[:, k * 128:(k + 1) * 128],
                                         W1[p, 2][:, :], start=True,
                                         stop=True)
                        nc.vector.tensor_copy(Pt[:, s0 + k, :], pps[:, :])
                hs = {}
                for p in ("Eg", "Sg", "Ln", "Mn"):
                    hps = ph.tile([128, 512], f32, tag="hps")
                    nc.tensor.matmul(hps[:, :wd], W1[p, 0][:, :],
                                     tabT[:, :wd], start=True, stop=True)
                    hsb = hp.tile([128, 512], bf16, tag="hsb")
                    nc.scalar.activation(hsb[:, :wd], hps[:, :wd], AF.Silu,
                                         bias=B1[p][:, :])
                    hs[p] = hsb
                spsum = ps.tile([8, 512], f32, tag="spsum")
                for j, p in enumerate(("Ln", "Mn", "Eg", "Sg")):
                    nc.tensor.matmul(spsum[:, :wd], W2[p][:, :],
                                     hs[p][:, :wd], start=(j == 0),
                                     stop=(j == 3))
                ssb = wp.tile([8, 512], f32, tag="ssb")
                nc.scalar.copy(ssb[:, :wd], spsum[:, :wd])
                for k in range(gw):
                    ptr8 = ps.tile([128, 8], f32, tag="tr")
                    nc.tensor.transpose(ptr8[:, :],
                                        ssb[:8, k * 128:(k + 1) * 128],
                                        ident_f[:8, :8])
                    nc.vector.tensor_add(gradsL[:, s0 + k, :], ptr8[:, :],
                                         b2n_t[:, :])

            # ---------------- phase 2: edge tiles ----------------
            win_psum = None
            for tg in range(NT // SBATCH):
                S4 = wp.tile([128, SBATCH * TPC, 8], f32, tag="S4")
                for ti in range(SBATCH):
                    t = tg * SBATCH + ti
                    ea_b = wp.tile([128, cfg.TILE], bf16, tag="ea")
                    nc.gpsimd.dma_start(
                        ea_b[:, :],
                        ea_T[:, t * cfg.TILE:(t + 1) * cfg.TILE])
                    six = wp.tile([128, cfg.TILE // 16], i16, tag="six")
                    nc.sync.dma_start(
                        six[:, :], sidx[:, t * cfg.TILE // 16:
                                        (t + 1) * cfg.TILE // 16])
                    rwrt = wp.tile([1, cfg.TILE], f32, tag="rwrt")
                    nc.sync.dma_start(
                        rwrt[:, :], rwr_in[:, t * cfg.TILE:
                                           (t + 1) * cfg.TILE])

                    stream = schedule[t * TPC][0]
                    gs = wp.tile([128, 1, cfg.TILE], bf16, tag="gs")
                    src = tab_rows[:, :]
                    if stream == 1:
                        src = tab_rows[:, cfg.SPLIT:]
                    nc.gpsimd.dma_gather(
                        gs[:, :, :], src, six[:, :], num_idxs=cfg.TILE,
                        num_idxs_reg=cfg.TILE, elem_size=H, transpose=True,
                        sbuf_tokens_per_rank=128,
                        sbuf_free_dim_per_rank=2 * H)

                    # receiver one-hot U^T [128n, 512e], one tile per
                    # window span (usually one span per tile)
                    wins = [schedule[t * TPC + k][1] for k in range(TPC)]
                    spans = []
                    for k, w_k in enumerate(wins):
                        if spans and spans[-1][0] == w_k:
                            spans[-1][2] = (k + 1) * 128
                        else:
                            spans.append([w_k, k * 128, (k + 1) * 128])
                    bc = pb.tile([128, cfg.TILE], f32, tag="bq")
                    nc.tensor.matmul(bc[:, :], ones_c[:, :], rwrt[:, :],
                                     start=True, stop=True)
                    UTs = []
                    for (w_s, c0, c1) in spans:
                        UT = wp.tile([128, cfg.TILE], bf16, tag="UT")
                        if len(spans) > 1:
                            nc.vector.memset(UT[:, :], 0.0)
                        nc.vector.tensor_scalar(UT[:, c0:c1], bc[:, c0:c1],
                                                iota_col[:, :], None,
                                                ALU.is_equal)
                        UTs.append((w_s, UT))
                    if DEBUG_TAPS and t == 0:
                        dU = wp.tile([128, cfg.TILE], f32, tag="dU")
                        nc.vector.tensor_copy(dU[:, :], UTs[0][1][:, :])
                        nc.sync.dma_start(dbg_UT[:, :], dU[:, :])

                    hs = {}
                    for p in ("Le", "Me"):
                        Pt = PLe if p == "Le" else PMe
                        hps = ph.tile([128, cfg.TILE], f32, tag="hps")
                        prev = nc.tensor.matmul(hps[:, :], W1[p, 0][:, :],
                                                ea_b[:, :], start=True,
                                                stop=False)
                        for (w_s, UT) in UTs:
                            cur = nc.tensor.matmul(hps[:, :], Pt[:, w_s, :],
                                                   UT[:, :], start=False,
                                                   stop=False)
                            tile.add_dep_helper(cur.ins, prev.ins, sync=False,
                                                reason="psum accum order")
                            prev = cur
                        cur = nc.tensor.matmul(hps[:, :], W1[p, 1][:, :],
                                               gs[:, 0, :], start=False,
                                               stop=True)
                        tile.add_dep_helper(cur.ins, prev.ins, sync=False,
                                            reason="psum accum order")
                        if DEBUG_TAPS and t == 0 and p == "Le":
                            dh = wp.tile([128, cfg.TILE], f32, tag="dh")
                            nc.vector.tensor_copy(dh[:, :], hps[:, :])
                            nc.sync.dma_start(dbg_h[:, :], dh[:, :])
                        hsb = hp.tile([128, cfg.TILE], bf16, tag="hsb")
                        nc.scalar.activation(hsb[:, :], hps[:, :], AF.Silu,
                                             bias=B1[p][:, :])
                        hs[p] = hsb
                    for p in ("Eg", "Sg"):
                        hps = ph.tile([128, cfg.TILE], f32, tag="hps")
                        nc.tensor.matmul(hps[:, :], W1[p, 0][:, :],
                                         gs[:, 0, :], start=True, stop=True)
                        hsb = hp.tile([128, cfg.TILE], bf16, tag="hsb")
                        nc.scalar.activation(hsb[:, :], hps[:, :], AF.Silu,
                                             bias=B1[p][:, :])
                        hs[p] = hsb
                    spsum = ps.tile([8, 512], f32, tag="spsum")
                    for j, p in enumerate(("Le", "Me", "Eg", "Sg")):
                        nc.tensor.matmul(spsum[:, :], W2[p][:, :],
                                         hs[p][:, :], start=(j == 0),
                                         stop=(j == 3))
                    ssb = wp.tile([8, 512], f32, tag="ssb")
                    nc.scalar.copy(ssb[:, :], spsum[:, :])
                    Tps = ps.tile([128, TPC, 8], f32, tag="tr")
                    for k in range(TPC):
                        nc.tensor.transpose(Tps[:, k, :],
                                            ssb[:8, k * 128:(k + 1) * 128],
                                            ident_f[:8, :8])
                    for k in range(TPC):
                        nc.vector.tensor_add(S4[:, ti * TPC + k, :],
                                             Tps[:, k, :], b2e_t[:, :])
                # term algebra, batched over SBATCH tiles
                NCH = SBATCH * TPC

                def col(v):
                    return S4[:, :, v:v + 1]
                u = wp.tile([128, NCH, 1], f32, tag="u")
                p1 = wp.tile([128, NCH, 1], f32, tag="p1")
                term = wp.tile([128, NCH, 2], f32, tag="term")
                nc.vector.tensor_mul(u[:, :, :], col(1), col(6))
                nc.vector.tensor_mul(p1[:, :, :], col(2), col(7))
                nc.vector.tensor_add(u[:, :, :], u[:, :, :], p1[:, :, :])
                nc.vector.tensor_mul(term[:, :, 0:1], col(1), u[:, :, :])
                nc.vector.tensor_mul(p1[:, :, :], col(0), col(5))
                nc.vector.tensor_sub(term[:, :, 0:1], term[:, :, 0:1],
                                     p1[:, :, :])
                nc.vector.tensor_mul(term[:, :, 1:2], col(2), u[:, :, :])
                nc.vector.tensor_mul(p1[:, :, :], col(0), col(4))
                nc.vector.tensor_add(term[:, :, 1:2], term[:, :, 1:2],
                                     p1[:, :, :])
                nc.vector.tensor_mul(p1[:, :, :], col(3), col(3))
                nc.vector.tensor_mul(p1[:, :, :], p1[:, :, :], col(7))
                nc.vector.tensor_add(term[:, :, 1:2], term[:, :, 1:2],
                                     p1[:, :, :])
                term_b = wp.tile([128, NCH, 2], bf16, tag="termb")
                nc.vector.tensor_copy(term_b[:, :, :], term[:, :, :])

                rwt = wp.tile([128, NCH], f32, tag="rwt")
                nc.sync.dma_start(rwt[:, :],
                                  rw_in[:, tg * NCH:(tg + 1) * NCH])
                for k in range(NCH):
                    s_, w_, st_, sp_ = schedule[tg * NCH + k]
                    U = wp.tile([128, cfg.WIN], bf16, tag="U")
                    nc.vector.tensor_scalar(U[:, :], iota_row[:, :cfg.WIN],
                                            rwt[:, k:k + 1], None,
                                            ALU.is_equal)
                    if st_:
                        win_psum = pw.tile([2, cfg.WIN], f32, tag="wps")
                    nc.tensor.matmul(win_psum[:, :], term_b[:, k, :],
                                     U[:, :], start=st_, stop=sp_)
                    if sp_:
                        a_sl = acc[:, w_ * cfg.WIN:(w_ + 1) * cfg.WIN]
                        nc.vector.tensor_add(a_sl, a_sl, win_psum[:, :])

            # ---------------- phase 3: finalize ----------------
            dzs = rp.tile([128, NST, 2], f32, tag="dzs")
            dEs = rp.tile([128, NST, 2], f32, tag="dEs")
            dSs = rp.tile([128, NST, 2], f32, tag="dSs")
            for (s0, gw) in groups:
                accT = ps.tile([128, 4, 2], f32, tag="tr")
                for k in range(gw):
                    nc.tensor.transpose(
                        accT[:, k, :],
                        acc[:, (s0 + k) * 128:(s0 + k + 1) * 128],
                        ident_f[:2, :2])
                G = gradsL[:, s0:s0 + gw, :]

                def gcol(v):
                    return G[:, :, v:v + 1]
                u = wp.tile([128, 4, 1], f32, tag="fu")
                p1 = wp.tile([128, 4, 1], f32, tag="fp1")
                us = u[:, :gw, :]
                p1s = p1[:, :gw, :]
                nc.vector.tensor_mul(us, gcol(1), gcol(6))
                nc.vector.tensor_mul(p1s, gcol(2), gcol(7))
                nc.vector.tensor_add(us, us, p1s)
                dz_sl = dzs[:, s0:s0 + gw, :]
                nc.vector.tensor_mul(dz_sl[:, :, 0:1], gcol(1), us)
                nc.vector.tensor_mul(p1s, gcol(0), gcol(5))
                nc.vector.tensor_sub(dz_sl[:, :, 0:1], dz_sl[:, :, 0:1], p1s)
                nc.vector.tensor_mul(dz_sl[:, :, 1:2], gcol(2), us)
                nc.vector.tensor_mul(p1s, gcol(0), gcol(4))
                nc.vector.tensor_add(dz_sl[:, :, 1:2], dz_sl[:, :, 1:2], p1s)
                nc.vector.tensor_mul(p1s, gcol(3), gcol(3))
                nc.vector.tensor_mul(p1s, p1s, gcol(7))
                nc.vector.tensor_add(dz_sl[:, :, 1:2], dz_sl[:, :, 1:2], p1s)
                nc.vector.tensor_sub(dz_sl, dz_sl, accT[:, :gw, :])
                dE_sl = dEs[:, s0:s0 + gw, :]
                nc.vector.tensor_mul(us, gcol(1), gcol(4))
                nc.vector.tensor_mul(p1s, gcol(2), gcol(5))
                nc.vector.tensor_add(us, us, p1s)
                nc.vector.tensor_mul(dE_sl[:, :, 0:1], gcol(1), us)
                nc.vector.tensor_mul(dE_sl[:, :, 1:2], gcol(2), us)
                nc.vector.tensor_mul(p1s, gcol(3), gcol(3))
                nc.vector.tensor_mul(p1s, p1s, gcol(5))
                nc.vector.tensor_add(dE_sl[:, :, 1:2], dE_sl[:, :, 1:2], p1s)
                dS_sl = dSs[:, s0:s0 + gw, :]
                nc.vector.tensor_mul(p1s, gcol(0), gcol(7))
                nc.vector.memset(dS_sl[:, :, 0:1], 0.0)
                nc.vector.tensor_sub(dS_sl[:, :, 0:1], dS_sl[:, :, 0:1], p1s)
                nc.vector.tensor_mul(dS_sl[:, :, 1:2], gcol(0), gcol(6))
            if DEBUG_TAPS:
                dP = rp.tile([128, NST, H], f32, tag="dP")
                nc.vector.tensor_copy(dP[:, :, :], PLe[:, :, :])
                nc.sync.dma_start(dbg_P[:, :, :], dP[:, :, :])
            nc.sync.dma_start(outs["dz"][:, :, :], dzs[:, :, :])
            nc.sync.dma_start(outs["dE"][:, :, :], dEs[:, :, :])
            nc.sync.dma_start(outs["dS"][:, :, :], dSs[:, :, :])
    nc.compile()
    return nc


# ----------------------------------------------------------------- entry ---
def kernel(**inputs):
    from concourse.bass_utils import run_bass_kernel_spmd

    cfg = Cfg()
    schedule, E_pad, node_pad, pci = host_prep(
        cfg, inputs["node_attr"], inputs["edge_index"], inputs["edge_attr"])
    w = pack_weights(cfg, inputs)

    nc = build_nc(cfg, schedule, E_pad)

    in_maps = []
    for i in range(cfg.NCORES):
        m = dict(node_pad=node_pad, b2row_e=w["b2row_e"],
                 b2row_n=w["b2row_n"])
        for p in ("Le", "Me", "Eg", "Sg", "Ln", "Mn"):
            m[p + "_W1"] = w[p + "_W1"]
            m[p + "_b1"] = w[p + "_b1"]
            m[p + "_W2p"] = w[p + "_W2p"]
        m.update(pci[i])
        in_maps.append(m)

    trace = os.environ.get("BASS_KERNEL_TRACE") == "1"
    r = run_bass_kernel_spmd(nc, in_maps, list(range(cfg.NCORES)),
                             trace=trace,
                             trace_cores=list(range(cfg.NCORES)) if trace
                             else None)
    if trace:
        print(f"HW exec time: {r.exec_time_ns} ns  "
              f"(mean {r.mean_exec_time_ns} ns, "
              f"max core {r.max_exec_time_core_id})")
    res = r.results

    dz = np.zeros((cfg.N, 2), np.float32)
    dE = np.zeros((cfg.N, 2), np.float32)
    dS = np.zeros((cfg.N, 2), np.float32)
    for i in range(cfg.NCORES):
        lo = i * cfg.NL
        hi = min((i + 1) * cfg.NL, cfg.N)
        n = hi - lo
        for nm, buf in (("dz_out", dz), ("dE_out", dE), ("dS_out", dS)):
            a = res[i][nm]  # [128, NST, 2]
            buf[lo:hi] = a.transpose(1, 0, 2).reshape(cfg.NL, 2)[:n]
    return (dz, dE[:, :, None], dS[:, :, None])
